# revision 33
# baseline (speedup 1.0000x reference)
"""Causal self-attention on 8 Trainium2 NeuronCores.

Sharding (data + head parallel): core c handles batch b = c // 4 and the
4 heads [4g, 4g+4) where g = c % 4.  Each core projects q/k/v for its
heads (weights pre-sliced + pre-transposed on host), runs causal
attention, then the 4 cores of each batch AllGather the per-head
attention outputs (hd-major fp16) and each computes a disjoint
256-channel column slice of the output projection.

Pipeline (v5, software-pipelined):
- x is DMA'd in 512-column slabs; weights in 128-col slices so the first
  projection matmul starts as early as possible.
- A filler queue interleaves projection / output-projection matmul
  groups between attention j-steps, keeping the in-order PE stream dense
  while ACT works through the exps.  Fillers borrow whichever of the two
  "at"/"pp" PSUM slots the live attention accumulator is NOT using.
- One AllGather per 512-query chunk (8 small collectives).  Out-proj
  groups are hosted >= 2 chunks after their gather fires (stage chain +
  collective is ~17us) so the PE stream never waits on a collective;
  only the last chunk's gather is on the tail.  The chunk order
  finishes (1,2),(0,2),(1,3),(0,3) so the final gathers interleave.
- Softmax reciprocal runs on the scalar engine as exp(-ln(x)) — the
  "natural_log_exp_and_others" ACT table holds ln and exp together, so
  there is no table switching and the slow iterative DVE reciprocal
  (which blocked the in-order DVE stream) is avoided.
- Output accumulates per 512-col region: first pair's group parks in
  SBUF f32, the closing group does fused add+cast-f16+DMA.
- fp16 data path, fp32 PSUM accumulation, f16 output (host casts).

Layouts per core:
  xT    (1024, 2048)  x[b].T                       (d on partitions)
  wqkT  (1024, 512)   [ (Wq[rows]/8).T | Wk[rows].T ]
  wvT   (1024, 256)   Wv[rows].T
  woT   (1024, 256)   Wo[rows].T with rows permuted to the AllGather
                      order: [pair p=0: rank r: heads 4r,4r+1] then
                      [pair p=1: rank r: heads 4r+2,4r+3]
  mask  (128, 128)    upper-triangular ones (k <= q)
  outT  (256, 2048)   out[b][:, cols].T  (f16)
"""

from collections import deque

import numpy as np

B, S, D, H = 2, 2048, 1024, 16
HD = D // H              # 64
NCORES = 8
GROUP = 4                # cores per batch
LHEADS = 4               # heads per core
LCH = LHEADS * HD        # 256 local channels
KT = D // 128            # 8 contraction tiles
ST = S // 128            # 16 sequence tiles
PAIRS = 2                # head pairs per core
CHUNK = 512              # q columns per attention pass
NCH = S // CHUNK         # 4

_CACHE = {}


def _f16(a):
    return np.ascontiguousarray(a, dtype=np.float16)


def _patch_act_tables():
    """Make Exp and Ln resolve to the single combined ACT table set
    ("natural_log_exp_and_others") so the kernel's exp+ln mix loads one
    table once instead of thrashing between the exp-only and ln sets
    (~1.5us per reload, dozens of reloads).  Set order/indices are
    preserved; only which set may serve exp/ln changes."""
    import concourse.bacc as bacc
    import concourse.mybir as mybir
    from concourse import hw_specs

    if getattr(_patch_act_tables, "done", False):
        return
    orig = hw_specs.get_activation_tables

    def patched(arch):
        tables = orig(arch)
        if "natural_log_exp_and_others" in tables:
            for name, funcs in tables.items():
                if name != "natural_log_exp_and_others":
                    funcs.discard(mybir.ActivationFunctionType.Exp)
                    funcs.discard(mybir.ActivationFunctionType.Ln)
        return tables

    bacc.get_activation_tables = patched
    _patch_act_tables.done = True


def _build():
    import concourse.bacc as bacc
    import concourse.mybir as mybir
    import concourse.tile as tile

    _patch_act_tables()

    f32 = mybir.dt.float32
    f16 = mybir.dt.float16
    Exp = mybir.ActivationFunctionType.Exp
    Ln = mybir.ActivationFunctionType.Ln

    nc = bacc.Bacc(num_devices=NCORES)
    xT = nc.dram_tensor("xT", [D, S], f16, kind="ExternalInput")
    wqkT = nc.dram_tensor("wqkT", [D, 2 * LCH], f16, kind="ExternalInput")
    wvT = nc.dram_tensor("wvT", [D, LCH], f16, kind="ExternalInput")
    woT = nc.dram_tensor("woT", [D, LCH], f16, kind="ExternalInput")
    # cols 0-127: identity; cols 128-255: causal bias (-30 where key > q)
    mask = nc.dram_tensor("mask", [128, 256], f16, kind="ExternalInput")
    outT = nc.dram_tensor("outT", [LCH, S], f16, kind="ExternalOutput")

    RG = [[0, 1, 2, 3], [4, 5, 6, 7]]

    with tile.TileContext(nc, num_cores=NCORES) as tc:
        with (
            tc.tile_pool(name="const", bufs=1) as const,
            tc.tile_pool(name="qkv", bufs=1) as qkv,
            tc.tile_pool(name="psum", bufs=1, space="PSUM") as psum,
            tc.tile_pool(name="dram", bufs=1, space="DRAM") as dram,
            tc.tile_pool(name="work", bufs=1) as work,
            tc.tile_pool(name="proj", bufs=1) as projp,
            tc.tile_pool(name="agp", bufs=1) as agp,
        ):
            mask_sb = const.tile([128, 256], f16)
            nc.sync.dma_start(mask_sb[:], mask[:])
            maskm = mask_sb[:, 0:128]
            ones4 = const.tile([128, LHEADS], f32)
            nc.vector.memset(ones4[:], 1.0)

            cc_in = [[dram.tile([128, CHUNK], f16, name=f"ccin{p}{c}")
                      for c in range(NCH)] for p in range(PAIRS)]
            cc_out = [[dram.tile([GROUP * 128, CHUNK], f16, name=f"ccout{p}{c}")
                       for c in range(NCH)] for p in range(PAIRS)]

            qt = qkv.tile([128, PAIRS, S], f16)
            kt = qkv.tile([128, PAIRS, S], f16)
            v = qkv.tile([128, ST, LHEADS, 65], f16)

            # ---------------- input loads (slab-major for x) ----------------
            wqk, wv, xt = [], [], []
            for k in range(KT):
                tw = projp.tile([128, 2 * LCH], f16, name=f"wqk{k}")
                nc.sync.dma_start(tw[:], wqkT[128 * k:128 * k + 128, :])
                wqk.append(tw)
            for k in range(KT):
                tx = projp.tile([128, NCH, CHUNK], f16, name=f"xt{k}")
                xt.append(tx)
            for s in range(NCH):
                for k in range(KT):
                    nc.sync.dma_start(
                        xt[k][:, s, :],
                        xT[128 * k:128 * k + 128, CHUNK * s:CHUNK * s + CHUNK])
                if s == 0:
                    for k in range(KT):
                        tv = projp.tile([128, LCH], f16, name=f"wv{k}")
                        nc.sync.dma_start(tv[:], wvT[128 * k:128 * k + 128, :])
                        wv.append(tv)
            wo = projp.tile([128, KT, LCH], f16)
            nc.sync.dma_start(wo[:], woT[:].rearrange("(k p) n -> p k n", p=128))

            # ---------------- op builders ----------------
            def qk_proj(m, s, tag):
                # m: 0,1 = q pair 0/1; 2,3 = k pair 0/1; s: 512-col slab
                dst = qt if m < 2 else kt
                pp = psum.tile([128, CHUNK], f32, tag=tag, name=f"qk{m}{s}")
                for k in range(KT):
                    nc.tensor.matmul(
                        pp[:], wqk[k][:, 128 * m:128 * m + 128], xt[k][:, s, :],
                        start=(k == 0), stop=(k == KT - 1))
                nc.vector.tensor_copy(
                    dst[:, m % 2, CHUNK * s:CHUNK * s + CHUNK], pp[:])

            def v_proj(j, tag):
                vps = psum.tile([128, LCH], f32, tag=tag, name=f"v{j}")
                for k in range(KT):
                    nc.tensor.matmul(
                        vps[:], xt[k][:, j // 4, 128 * (j % 4):128 * (j % 4) + 128],
                        wv[k][:], start=(k == 0), stop=(k == KT - 1))
                nc.vector.tensor_copy(
                    v[:, j, :, 64:65], ones4[:].rearrange("p (h o) -> p h o", o=1))
                nc.vector.tensor_copy(
                    v[:, j, :, 0:64], vps[:].rearrange("p (h e) -> p h e", h=LHEADS))

            agt = {}
            oacc = [agp.tile([128, 2 * CHUNK], f32, name=f"oacc{ct}")
                    for ct in range(2)]
            ofirst = {}

            def fetch_ag(ph, c):
                """Prefetch the gathered chunk (ph, c) into SBUF."""
                ts = []
                for r in range(GROUP):
                    t = agp.tile([128, CHUNK], f16, tag=f"ag{ph}{c % 2}{r}",
                                 bufs=1, name=f"ag{ph}{c}{r}")
                    nc.sync.dma_start(
                        t[:], cc_out[ph][c][128 * r:128 * r + 128, :])
                    ts.append(t)
                agt[(ph, c)] = ts

            def out_group(ph, c, ct, tag):
                """4-matmul contribution of pair ph's gathered chunk c to
                output rows [128ct, 128ct+128).  First pair per region parks
                in SBUF f32; the closing pair does fused add+f16+DMA."""
                cc = c % 2
                hb = c // 2
                pp = psum.tile([128, CHUNK], f32, tag=tag, name=f"og{ph}{c}{ct}")
                for k4 in range(GROUP):
                    nc.tensor.matmul(
                        pp[:], wo[:, 4 * ph + k4, 128 * ct:128 * ct + 128],
                        agt[(ph, c)][k4][:],
                        start=(k4 == 0), stop=(k4 == GROUP - 1))
                key = (hb, ct, cc)
                if key not in ofirst:
                    ofirst[key] = True
                    nc.vector.tensor_copy(
                        oacc[ct][:, 512 * cc:512 * cc + 512], pp[:])
                else:
                    ot = agp.tile([128, CHUNK], f16, tag=f"ot{ct}", bufs=2,
                                  name=f"ot{ph}{c}{ct}")
                    nc.vector.tensor_add(
                        ot[:], pp[:], oacc[ct][:, 512 * cc:512 * cc + 512])
                    nc.sync.dma_start(
                        outT[128 * ct:128 * ct + 128,
                             1024 * hb + 512 * cc:1024 * hb + 512 * cc + 512],
                        ot[:])

            # ---------------- filler machinery ----------------
            fillers = deque()
            free_tag = ["pp"]

            def emit(k):
                for _ in range(min(k, len(fillers))):
                    fillers.popleft()(free_tag[0])

            def drain():
                emit(len(fillers))

            def stage_chunk(p, c, attps):
                """Normalize chunk c's accumulators, ship to the collective
                buffer, and AllGather this chunk.  Reciprocal of the
                denominator row runs on ACT as exp(-ln(x))."""
                asb = work.tile([65, 2 * CHUNK], f32, tag="asb", bufs=2,
                                name=f"asb{p}{c}")
                nc.vector.tensor_copy(asb[:], attps[:])
                t1 = work.tile([65, 2 * CHUNK], f32, tag="t1", bufs=2,
                               name=f"t1{p}{c}")
                nc.scalar.activation(t1[64:65, :], attps[64:65, :], Ln)
                rc = work.tile([65, 2 * CHUNK], f32, tag="rc", bufs=2,
                               name=f"rc{p}{c}")
                nc.scalar.activation(rc[64:65, :], t1[64:65, :], Exp, scale=-1.0)
                rc0 = work.tile([1, 2 * CHUNK], f32, tag="rc0", bufs=2,
                                name=f"rc0{p}{c}")
                nc.sync.dma_start(rc0[0:1, :], rc[64:65, :])
                bc = work.tile([64, 2 * CHUNK], f32, tag="bc", bufs=2,
                               name=f"bc{p}{c}")
                nc.gpsimd.partition_broadcast(bc[:], rc0[0:1, :])
                ao = work.tile([64, 2 * CHUNK], f16, tag="ao", bufs=2,
                               name=f"ao{p}{c}")
                nc.vector.tensor_mul(ao[:, :], asb[0:64, :], bc[:, :])
                nc.sync.dma_start(
                    cc_in[p][c][:].rearrange("(h p) n -> p h n", h=2),
                    ao[:].rearrange("p (h n) -> p h n", h=2))
                nc.gpsimd.collective_compute(
                    "AllGather", mybir.AluOpType.bypass, replica_groups=RG,
                    ins=[cc_in[p][c][:]], outs=[cc_out[p][c][:]])

            cidx = [0]

            def attn_chunk(p, c, host=1, mid=(), late=()):
                q0 = CHUNK * c
                nj = 4 * c + 4
                tag = "at" if cidx[0] % 2 == 0 else "pp"
                cidx[0] += 1
                free_tag[0] = "pp" if tag == "at" else "at"
                attps = psum.tile([65, 2 * CHUNK], f32, tag=tag,
                                  name=f"att{p}{c}")

                def attend(j, ex, n, off):
                    for h in range(2):
                        nc.tensor.matmul(
                            attps[:, 512 * h + off:512 * h + CHUNK],
                            v[:, j, 2 * p + h, :],
                            ex[:, 512 * h:512 * h + n],
                            start=(j == 0), stop=(j == nj - 1))

                pend = None
                for j in range(nj):
                    qs = max(q0, 128 * j)
                    n = q0 + CHUNK - qs
                    off = qs - q0
                    sc = psum.tile([128, 1024], f32, tag="sc", bufs=2,
                                   name=f"sc{p}{c}{j}")
                    for h in range(2):
                        pb = 64 * h
                        nc.tensor.matmul(
                            sc[:, 512 * h:512 * h + n],
                            kt[pb:pb + 64, p, 128 * j:128 * j + 128],
                            qt[pb:pb + 64, p, qs:qs + n],
                            start=True, stop=True)
                    ex = work.tile([128, 1024], f16, tag="ex", bufs=4,
                                   name=f"ex{p}{c}{j}")
                    nc.scalar.activation(
                        ex[:].rearrange("q (t x) -> q t x", t=2)[:, :, 0:n],
                        sc[:].rearrange("q (t x) -> q t x", t=2)[:, :, 0:n],
                        Exp)
                    if qs == 128 * j:  # diagonal tile: causal mask
                        for h in range(2):
                            nc.vector.tensor_mul(
                                ex[:, 512 * h:512 * h + 128],
                                ex[:, 512 * h:512 * h + 128], maskm[:])
                    # attends run one j behind the scores/exp so the PE
                    # stream never sits on the freshest exp
                    if pend is not None:
                        attend(*pend)
                    pend = (j, ex, n, off)
                    if j == max(0, nj - 6):
                        for fn in mid:
                            fn(free_tag[0])
                    if j == nj - 2:
                        for fn in late:
                            fn(free_tag[0])
                    if j < nj - 1:
                        emit(host)

                attend(*pend)
                stage_chunk(p, c, attps)
                free_tag[0] = tag

            # ---------------- schedule ----------------
            # slab-0 head: just enough for chunk (0,0) to start
            qk_proj(0, 0, "at")
            qk_proj(2, 0, "pp")
            v_proj(0, "at")
            v_proj(1, "pp")
            v_proj(2, "at")
            v_proj(3, "pp")

            def push_slab(s):
                fillers.append(lambda t, s=s: qk_proj(0, s, t))
                fillers.append(lambda t, s=s: qk_proj(2, s, t))
                for j in range(4 * s, 4 * s + 4):
                    fillers.append(lambda t, j=j: v_proj(j, t))
                fillers.append(lambda t, s=s: qk_proj(1, s, t))
                fillers.append(lambda t, s=s: qk_proj(3, s, t))

            def push_groups(ph, c):
                for ct in range(2):
                    fillers.append(
                        lambda t, ph=ph, c=c, ct=ct: out_group(ph, c, ct, t))

            fillers.append(lambda t: qk_proj(1, 0, t))
            fillers.append(lambda t: qk_proj(3, 0, t))
            push_slab(1)

            attn_chunk(0, 0, host=3)
            attn_chunk(1, 0, host=2)
            push_slab(2)
            attn_chunk(0, 1, host=2)
            push_slab(3)
            fetch_ag(0, 0)
            push_groups(0, 0)
            attn_chunk(1, 1, host=2)
            fetch_ag(1, 0)
            push_groups(1, 0)
            attn_chunk(1, 2, host=1)
            drain()
            fetch_ag(0, 1)
            push_groups(0, 1)
            attn_chunk(0, 2, host=1)
            drain()
            fetch_ag(1, 1)
            push_groups(1, 1)
            attn_chunk(1, 3, host=1)
            drain()
            fetch_ag(1, 2)
            push_groups(1, 2)
            fetch_ag(0, 2)
            push_groups(0, 2)
            attn_chunk(0, 3, host=1,
                       mid=(lambda t: fetch_ag(1, 3),),
                       late=(lambda t: out_group(1, 3, 0, t),
                             lambda t: out_group(1, 3, 1, t)))
            drain()

            # tail: last gather + closing output groups
            fetch_ag(0, 3)
            out_group(0, 3, 0, "at")
            out_group(0, 3, 1, "pp")

    nc.compile()
    return nc


def _gather_perm():
    """d-channel permutation matching the AllGather layout."""
    perm = []
    for p in range(PAIRS):
        for r in range(GROUP):
            for h in range(2):
                head = 4 * r + 2 * p + h
                perm.extend(range(HD * head, HD * head + HD))
    return np.array(perm)


def _shard_inputs(x, Wq, Wk, Wv, Wo):
    x = np.asarray(x, dtype=np.float32)
    Wq = np.asarray(Wq, dtype=np.float32)
    Wk = np.asarray(Wk, dtype=np.float32)
    Wv = np.asarray(Wv, dtype=np.float32)
    Wo = np.asarray(Wo, dtype=np.float32)
    mask = np.concatenate(
        [np.triu(np.ones((128, 128), dtype=np.float16)),
         np.zeros((128, 128), dtype=np.float16)], axis=1)
    perm = _gather_perm()
    in_maps = []
    for c in range(NCORES):
        b, g = c // GROUP, c % GROUP
        rows = slice(LCH * g, LCH * g + LCH)
        in_maps.append({
            "xT": _f16(x[b].T),
            "wqkT": _f16(np.concatenate([Wq[rows] / 8.0, Wk[rows]], axis=0).T),
            "wvT": _f16(Wv[rows].T),
            "woT": _f16(Wo[rows].T[perm, :]),
            "mask": mask,
        })
    return in_maps


def kernel(x, Wq, Wk, Wv, Wo):
    from concourse.bass_utils import run_bass_kernel_spmd

    if "nc" not in _CACHE:
        _CACHE["nc"] = _build()
    nc = _CACHE["nc"]
    in_maps = _shard_inputs(x, Wq, Wk, Wv, Wo)
    res = run_bass_kernel_spmd(nc, in_maps, core_ids=list(range(NCORES)))
    _CACHE["last_results"] = res
    out = np.empty((B, S, D), dtype=np.float32)
    for c in range(NCORES):
        b, g = c // GROUP, c % GROUP
        out[b][:, LCH * g:LCH * g + LCH] = res.results[c]["outT"].T.astype(
            np.float32)
    return out
